# revision 15
# baseline (speedup 1.0000x reference)
"""NT-Xent / SimCLR contrastive loss on 8 Trainium2 NeuronCores (Bass/Tile).

Problem: zi, zj [4096, 512] f32 -> scalar loss.
  reps = concat(zi, zj)            [8192, 512]
  rn   = reps / max(||reps||, 1e-8)
  sim  = rn @ rn.T                 [8192, 8192]
  pos_i  = sim[i, (i+B) mod 2B]
  denom_i = sum_{j != i} exp(sim_ij / tau)
  loss = mean(-pos/tau + log(denom))

Sharding (per the hint, each device starts from its shard of the
normalized reps): the host normalizes + shards (the "each device holds
its row block of normalized reps" initial state), each core computes its
block-symmetric share of the similarity GEMM and the exp/partial-sum
reductions, and the host performs the final cross-core assembly + log +
mean (the scalar all-reduce).

Symmetric decomposition (identical to the proven baseline): core c owns
rows [c*1024, (c+1)*1024) and computes
  - GEMM-A: its rows x column blocks {c, c+1, c+2, c+3} (mod 8), 4096 cols
  - two antipodal quadrants vs block b=(c+4)%8 (512 cols per row half).
Row sums of exp come from the ACT accumulator; column sums of exp (the
mirrored row-partials destined for other cores' rows) are accumulated on
the vector engine into a [128, 4096] buffer and shipped to the host,
which folds the 128 partitions and assembles denom from all partials.

Speed: operands are fp8e4 (16*rn, exact power-of-2 scale) so the PE runs
DoubleRow perf mode (2 fp8 MACs/PE/cycle, K=256 per pass); exp tiles are
[128, 2048] (4 PSUM banks) to amortize ACT access + accumulator-read
overhead; the 8 antipodal quadrants are packed 4-per-PSUM-tile so all 8
cost only two ACT instructions, with their row sums taken on the DVE.
PSUM holds 256*sim; the ACT exp scale folds 1/256.  Self-similarity is
cancelled exactly: the own-diagonal is extracted from PSUM pre-exp and
re-exp'd with the same scale on the same LUT, so rowsum - selfexp is
exact (selfexp ~ e^{1/tau} ~ 1.6e6 vs denom ~ 1e4 - approximate
cancellation would be catastrophic).  The T0 exp tile (which contains
the own block) stays f32 so no rounding sits between the row-sum
accumulator and the extracted diagonal.
"""

import sys

for _p in ("/opt/trn_rl_repo",):
    if _p not in sys.path:
        sys.path.insert(0, _p)

from contextlib import ExitStack

import ml_dtypes
import numpy as np

TAU = 0.07
B, D = 4096, 512
NCORES = 8
ROWS = 2 * B              # 8192
RPC = ROWS // NCORES      # 1024 rows per core
NM = RPC // 128           # 8 m-tiles per core
KC = D // 128             # 4 k-subtiles of 128
CTOT = 5 * RPC            # 5120 GEMM columns per core
NA = 4 * RPC              # 4096 GEMM-A columns
CJ = 4096                 # colacc width (cols 1024..5120)
FP8S = 16.0               # operand scale (power of 2, exact in fp8)
SCALE = 1.0 / (FP8S * FP8S * TAU)   # ACT exp scale on PSUM values

_prog_cache = {}


def _build_program():
    import concourse.bacc as bacc
    import concourse.tile as tile
    import concourse.mybir as mybir
    import bass_rust

    dt = mybir.dt
    Alu = mybir.AluOpType
    Act = mybir.ActivationFunctionType
    DR = mybir.MatmulPerfMode.DoubleRow

    nc = bacc.Bacc("TRN2", target_bir_lowering=False, debug=False,
                   enable_asserts=False, num_devices=NCORES)

    rnT_in = nc.dram_tensor("rnT", [128, KC, CTOT], dt.float8e4,
                            kind="ExternalInput").ap()
    ident_f32 = nc.dram_tensor("ident_f32", [128, 128], dt.float32,
                               kind="ExternalInput").ap()
    out = nc.dram_tensor("out", [128, 16], dt.float32,
                         kind="ExternalOutput").ap()
    cacc_out = nc.dram_tensor("cacc_out", [128, CJ], dt.bfloat16,
                              kind="ExternalOutput").ap()

    with tile.TileContext(nc) as tc, ExitStack() as ctx:
        const = ctx.enter_context(tc.tile_pool(name="const", bufs=1))
        persist = ctx.enter_context(tc.tile_pool(name="persist", bufs=1))
        e0p = ctx.enter_context(tc.tile_pool(name="e0p", bufs=3))
        e1p = ctx.enter_context(tc.tile_pool(name="e1p", bufs=3))
        scrp = ctx.enter_context(tc.tile_pool(name="scrp", bufs=2))
        smallp = ctx.enter_context(tc.tile_pool(name="smallp", bufs=4))
        ps = ctx.enter_context(tc.tile_pool(name="ps", bufs=2,
                                            space="PSUM"))

        i32 = const.tile([128, 128], dt.float32, tag="i32")

        # Hoist the ACT table load off the critical path: a throwaway
        # activation at the top of the ACT program makes bacc place the
        # (1.3us) table load in the startup window instead of in front of
        # the first real exp.
        warm_in = smallp.tile([128, 8], dt.float32, tag="warm_in")
        warm_out = smallp.tile([128, 8], dt.float32, tag="warm_out")
        nc.vector.tensor_scalar(out=warm_in[:], in0=warm_in[:], scalar1=0.0,
                                scalar2=None, op0=Alu.mult)
        nc.scalar.activation(warm_out[:], warm_in[:], Act.Exp, scale=SCALE)

        # rnT is split into one tile per 256KB piece (k-subtile-pair,
        # 512-col window-half) so Tile's per-tile write tracking lets each
        # Ldweights/matmul start as soon as ITS piece lands, and the
        # pieces round-robin across all three DMA queues (SP + ACT HWDGE,
        # GpSimd SWDGE) in consumption order: the input load is
        # HBM-bandwidth-bound (~8.5us for 2.6MB), so the GEMM chases the
        # DMA wavefront instead of waiting for it to finish.
        rnP = {}
        for cp in range(2):
            for w in range(5):
                for h in range(2):
                    tl = persist.tile([128, 2 * 512], dt.float8e4,
                                      tag=f"rn{cp}{w}{h}")
                    rnP[(cp, w, h)] = tl[:].rearrange("p (c w) -> p c w",
                                                      c=2)
        colacc = persist.tile([128, CJ], dt.bfloat16, tag="colacc")
        rs = persist.tile([128, 16], dt.float32, tag="rs")
        dv = persist.tile([128, NM], dt.float32, tag="dv")
        qsum = persist.tile([128, NM], dt.float32, tag="qsum")
        outbuf = persist.tile([128, 16], dt.float32, tag="outbuf")

        dma_in = {}
        _order = ([(0, w, h) for w in range(4) for h in range(2)]
                  + [(1, w, h) for w in range(4) for h in range(2)]
                  + [(0, 4, 0), (0, 4, 1), (1, 4, 0), (1, 4, 1)])
        _queues = [nc.sync, nc.scalar, nc.gpsimd]
        for k, (cp, w, h) in enumerate(_order):
            dma_in[(cp, w, h)] = _queues[k % 3].dma_start(
                rnP[(cp, w, h)],
                rnT_in[:, 2 * cp:2 * cp + 2,
                       w * 1024 + h * 512:w * 1024 + (h + 1) * 512])
            if k == 8:
                # ident needed by the first diag STT (~15us in)
                nc.gpsimd.dma_start(i32[:], ident_f32[:])

        def sdep(inst, dma):
            inst.ins.add_dependency(dma.ins.name,
                                    bass_rust.DependencyInfo.SYNC_ONLY)

        # PSUM WAR edges are missing from Tile's tracker: a slot-recycling
        # matmul (start=True resets the region) must wait for the previous
        # occupant's readers (exp / diag STT). Track readers per pool slot.
        ps_readers = {}
        mv_dep_done = set()

        def war_dep(mm, readers):
            for rname in readers:
                mm.ins.add_dependency(rname, bass_rust.DependencyInfo.SYNC_ONLY)

        # The matmul MOVING-operand read has no tracked edge to the DMA
        # that writes it (only the Ldweights/stationary read is tracked);
        # add one manual edge per piece on its first reader.
        def mv_dep(mm, cp, w, h):
            if (cp, w, h) not in mv_dep_done:
                mv_dep_done.add((cp, w, h))
                sdep(mm, dma_in[(cp, w, h)])

        def stat_ap(c2, t):
            return rnP[(c2, 0, t // 4)][:, :, (t % 4) * 128:
                                        (t % 4) * 128 + 128]

        tilectr = 0
        selfexp_t = []

        def gemm_a_tile(t, last=False):
            """m-tile t: T0 = [own|+1] cols 0..2048, T1 = [+2|+3]."""
            nonlocal tilectr
            slot0 = tilectr % 2
            slot1 = (tilectr + 1) % 2
            ps0 = ps.tile([128, 2048], dt.float32, tag="ps")
            ps1 = ps.tile([128, 2048], dt.float32, tag="ps")
            for c2 in range(2):
                stat = stat_ap(c2, t)
                for half, pst, slot in ((0, ps0, slot0), (1, ps1, slot1)):
                    for piece in range(4):
                        w = half * 2 + piece // 2
                        h = piece % 2
                        mm = nc.tensor.matmul(
                            pst[:, piece * 512:(piece + 1) * 512], stat,
                            rnP[(c2, w, h)][:, :, :],
                            start=(c2 == 0), stop=(c2 == 1), perf_mode=DR)
                        if c2 == 0 and piece == 0:
                            war_dep(mm, ps_readers.get(slot, ()))
                        mv_dep(mm, c2, w, h)
            # T0: own-diag extraction (exact self-exclusion) + f32 exp
            scr = scrp.tile([128, 128], dt.float32, tag="scrd")
            stt = nc.vector.scalar_tensor_tensor(
                out=scr[:], in0=ps0[:, t * 128:(t + 1) * 128], scalar=1.0,
                in1=i32[:], op0=Alu.mult, op1=Alu.mult,
                accum_out=dv[:, t:t + 1])
            e0 = e0p.tile([128, 2048], dt.float32, tag="e0")
            ex0 = nc.scalar.activation(e0[:], ps0[:], Act.Exp, scale=SCALE,
                                       accum_out=rs[:, t:t + 1])
            ps_readers[slot0] = [stt.ins.name, ex0.ins.name]
            if last:
                # all dv columns are final once the last T0 diag is out:
                # emit selfexp before the last T1 exp to shorten the tail
                se = smallp.tile([128, NM], dt.float32, tag="selfexp")
                nc.scalar.activation(se[:], dv[:], Act.Exp, scale=SCALE)
                selfexp_t.append(se)
            # colacc block +1 (cols 1024..2048 -> colacc 0..1024)
            if t == 0:
                nc.vector.tensor_scalar(out=colacc[:, 0:1024],
                                        in0=e0[:, 1024:2048], scalar1=0.0,
                                        scalar2=None, op0=Alu.add)
            else:
                nc.vector.tensor_add(colacc[:, 0:1024], colacc[:, 0:1024],
                                     e0[:, 1024:2048])
            if last:
                # block +1 colacc region final; overlaps the last T1 exp
                nc.sync.dma_start(cacc_out[:, 0:1024], colacc[:, 0:1024])
            # T1: bf16 exp, colacc blocks +2/+3
            e1 = e1p.tile([128, 2048], dt.bfloat16, tag="e1")
            ex1 = nc.scalar.activation(e1[:], ps1[:], Act.Exp, scale=SCALE,
                                       accum_out=rs[:, 8 + t:9 + t])
            ps_readers[slot1] = [ex1.ins.name]
            if t == 0:
                nc.vector.tensor_scalar(out=colacc[:, 1024:3072],
                                        in0=e1[:], scalar1=0.0,
                                        scalar2=None, op0=Alu.add)
            else:
                nc.vector.tensor_add(colacc[:, 1024:3072],
                                     colacc[:, 1024:3072], e1[:])
            tilectr += 2

        def quad_tile(qt):
            """antipodal quadrants for m-tiles 4qt..4qt+3, packed in one
            PSUM tile; row sums on the DVE, colacc region final after."""
            nonlocal tilectr
            slot = tilectr % 2
            psq = ps.tile([128, 2048], dt.float32, tag="ps")
            for c2 in range(2):
                for i in range(4):
                    t = qt * 4 + i
                    h = 0 if t < 4 else 1
                    mm = nc.tensor.matmul(
                        psq[:, i * 512:(i + 1) * 512], stat_ap(c2, t),
                        rnP[(c2, 4, h)][:, :, :],
                        start=(c2 == 0), stop=(c2 == 1), perf_mode=DR)
                    if c2 == 0 and i == 0:
                        war_dep(mm, ps_readers.get(slot, ()))
                    mv_dep(mm, c2, 4, h)
            readers = []
            for i in range(4):
                t = qt * 4 + i
                scr = scrp.tile([128, 128], dt.float32, tag="scrd")
                stt = nc.vector.scalar_tensor_tensor(
                    out=scr[:], in0=psq[:, i * 512 + (t % 4) * 128:
                                        i * 512 + (t % 4) * 128 + 128],
                    scalar=1.0, in1=i32[:], op0=Alu.mult, op1=Alu.mult,
                    accum_out=outbuf[:, 8 + t:9 + t])
                readers.append(stt.ins.name)
            eq = e1p.tile([128, 2048], dt.bfloat16, tag="e1")
            exq = nc.scalar.activation(eq[:], psq[:], Act.Exp, scale=SCALE)
            readers.append(exq.ins.name)
            ps_readers[slot] = readers
            # row sums of the 4 quads on the DVE (one op)
            nc.vector.reduce_sum(qsum[:, qt * 4:qt * 4 + 4],
                                 eq[:].rearrange("p (a w) -> p a w", a=4),
                                 axis=mybir.AxisListType.X)
            # colacc quad region 3072+qt*512 .. 3584+qt*512
            creg = slice(3072 + qt * 512, 3584 + qt * 512)
            for i in range(4):
                esub = eq[:, i * 512:(i + 1) * 512]
                if i == 0:
                    nc.vector.tensor_scalar(out=colacc[:, creg], in0=esub,
                                            scalar1=0.0, scalar2=None,
                                            op0=Alu.add)
                else:
                    nc.vector.tensor_add(colacc[:, creg], colacc[:, creg],
                                         esub)
            tilectr += 1
            # quad colacc region is final: stream it out early
            nc.gpsimd.dma_start(cacc_out[:, creg], colacc[:, creg])

        # Quad phases interleave mid-GEMM so their exp/colacc/reduce work
        # and output DMAs overlap GEMM-A instead of forming a tail.
        for t in (0, 1, 2, 3):
            gemm_a_tile(t)
        quad_tile(0)
        for t in (4, 5, 6):
            gemm_a_tile(t)
        quad_tile(1)
        gemm_a_tile(7, last=True)

        # ---- epilogue ----
        nc.gpsimd.dma_start(cacc_out[:, 1024:2048], colacc[:, 1024:2048])
        nc.scalar.dma_start(cacc_out[:, 2048:3072], colacc[:, 2048:3072])
        rsum = smallp.tile([128, NM], dt.float32, tag="rsum")
        nc.vector.tensor_add(rsum[:], rs[:, 0:8], rs[:, 8:16])
        nc.vector.tensor_add(rsum[:], rsum[:], qsum[:])
        nc.vector.tensor_sub(outbuf[:, 0:8], rsum[:], selfexp_t[0][:])
        nc.sync.dma_start(out[:], outbuf[:])

    # Pin bacc's activation-table choice to the one table holding Exp (and
    # Ln/Copy) so exactly one ACT table load is emitted.
    import concourse.bacc as bacc_mod
    _orig_tables = bacc_mod.get_activation_tables

    def _only_lnexp(arch):
        keep = "natural_log_exp_and_others"
        return {k: (v if k == keep else set())
                for k, v in _orig_tables(arch).items()}

    bacc_mod.get_activation_tables = _only_lnexp
    try:
        nc.compile()
    finally:
        bacc_mod.get_activation_tables = _orig_tables
    return nc


def _col_rows(c):
    """Global row indices of core c's 5120 GEMM columns, in rnT order."""
    b = (c + 4) % NCORES
    idxs = [np.arange(((c + d) % NCORES) * RPC, ((c + d) % NCORES + 1) * RPC)
            for d in range(4)]
    if c < 4:
        q = np.arange(b * RPC, (b + 1) * RPC)
    else:
        q = np.concatenate([np.arange(b * RPC + 512, (b + 1) * RPC),
                            np.arange(b * RPC, b * RPC + 512)])
    idxs.append(q)
    return np.concatenate(idxs)


def _host_inputs(zi, zj):
    reps = np.concatenate([np.asarray(zi, np.float64),
                           np.asarray(zj, np.float64)], axis=0)
    norms = np.maximum(np.linalg.norm(reps, axis=1, keepdims=True), 1e-8)
    rn8 = (FP8S * reps / norms).astype(np.float32).astype(
        ml_dtypes.float8_e4m3)                              # [8192, 512]
    ident_f32 = np.eye(128, dtype=np.float32)
    in_maps = []
    for c in range(NCORES):
        xt = rn8[_col_rows(c)].T                            # [512, 5120]
        rnT = np.ascontiguousarray(
            xt.reshape(KC, 128, CTOT).transpose(1, 0, 2))   # [128, 4, 5120]
        in_maps.append({"rnT": rnT, "ident_f32": ident_f32})
    return in_maps


def _postprocess(results):
    denom = np.zeros(ROWS, np.float64)
    pos = np.zeros(ROWS, np.float64)
    for c in range(NCORES):
        o = np.asarray(results[c]["out"], np.float64)        # [128, 16]
        ca = np.asarray(results[c]["cacc_out"], np.float64)  # [128, 4096]
        cr = _col_rows(c)
        for t in range(NM):
            rows = slice(c * RPC + t * 128, c * RPC + (t + 1) * 128)
            denom[rows] += o[:, t]
        # colsum partials: fold partitions, scatter to owning rows
        colsum = ca.sum(axis=0)                              # [4096]
        np.add.at(denom, cr[1024:], colsum)
        if c < 4:
            # PSUM diag = 256 * sim
            opos = o[:, 8:16].T.reshape(-1) / (FP8S * FP8S)  # [1024]
            rows = np.arange(c * RPC, (c + 1) * RPC)
            pos[rows] = opos
            pos[cr[4096:]] = opos
    loss = np.mean(-pos / TAU + np.log(denom))
    return np.asarray(loss, dtype=np.float32)


def kernel(zi, zj, _trace=False):
    from concourse.bass_utils import run_bass_kernel_spmd

    if "nc" not in _prog_cache:
        _prog_cache["nc"] = _build_program()
    nc = _prog_cache["nc"]
    in_maps = _host_inputs(zi, zj)
    res = run_bass_kernel_spmd(nc, in_maps, list(range(NCORES)),
                               trace=_trace)
    _prog_cache["last_result"] = res
    return _postprocess(res.results)
